# revision 29
# baseline (speedup 1.0000x reference)
"""Multi-head attention (RoPE + causal softmax) Trainium2 Bass kernel.

Sharding: 8 cores = 4 batches x 2 head-groups (tensor-parallel over heads).
Each core computes, for its (batch b, head-group g):
    Q/K/V projections for its 8 heads, RoPE, causal attention, and a
    partial output projection with its 512-row slice of W_O^T.
Host sums the two partial outputs per batch.

v2 restructure (vs the 811us baseline):
  - RoPE rotate-halves now: ACT copy PSUM->SBUF, 4 SBUF->SBUF DMA
    partition-block swaps, then 2 DVE muls + 1 add. (The old gpsimd
    copies were 1.86us each and serialized the projection phase.)
  - V stays in SBUF as fp16 with the ones-column baked in (layout
    [128, head, kblock, 65]); no DRAM round trip.
  - P (exp scores) in fp16: exp max here is ~13 so fp16 is safe and
    keeps ~2e-4 accuracy; halves mask-mul cost and pt SBUF footprint.
  - Attention is t-outer / head-inner, so attention on query-tile t only
    depends on projection pass t: the Tile scheduler overlaps proj pass
    t+1 with attention tile t, keeping the PE dense (HAM stays warm).
  - Normalization inline per (head, t): reciprocal straight from the
    PV PSUM ones-row, broadcast via a tiny ones-matmul.
  - Output projection runs per query-tile right after its 8 heads.
"""

import os
import sys
import types

import numpy as np

D_MODEL = 1024
NUM_HEADS = 16
HEAD_DIM = 64
THETA = 10000.0
BATCH = 4
SEQ = 2048
N_CORES = 8
HPC = 8          # heads per core
NCHUNK = HPC // 2  # 128-row chunks of the per-core 512 head dims
NQT = SEQ // 512   # 512-wide seq tiles
NSC = SEQ // 128   # 128-row seq chunks
KD = D_MODEL // 128  # contraction chunks for projections


# ---------------------------------------------------------------------------
# environment shims (axon container: missing antenv.axon_hooks; walrus here
# supports only 1 sync-wait per instruction)
# ---------------------------------------------------------------------------
def _install_axon_hooks():
    import antenv

    if hasattr(antenv, "axon_hooks"):
        return
    mod = types.ModuleType("antenv.axon_hooks")
    _hook = [None]
    mod.set_axon_ntff_profile_hook = lambda h: _hook.__setitem__(0, h)
    mod.get_axon_ntff_profile_hook = lambda: _hook[0]
    sys.modules["antenv.axon_hooks"] = mod
    antenv.axon_hooks = mod
    try:
        from trn_agent_boot.trn_boot import _ntff_profile_via_ctypes

        mod.set_axon_ntff_profile_hook(
            _ntff_profile_via_ctypes("/opt/axon/libaxon_pjrt.so")
        )
    except Exception:
        pass


def _install_drain_patch():
    import concourse.mybir as mybir
    import concourse.tile as tilemod

    if getattr(tilemod.TileContext, "_drain_patch_installed", False):
        return

    def _drain_and_barrier(self, tick_clock, wait_clock):
        carrier = self.nc.sync.nop(nofuse=True)
        wait_clock.add_sem_waits(
            carrier.ins, tilemod.ScopedClock({None: tick_clock.global_clock})
        )
        si = carrier.ins.sync_info
        if si is not None and si.on_wait and len(si.on_wait) > 1:
            waits = list(si.on_wait)
            carrier.ins.sync_info = mybir.SyncInfo(
                on_wait=[waits[0]], on_update=list(si.on_update or [])
            )
            for w in waits[1:]:
                nop = self.nc.sync.nop(nofuse=True)
                nop.ins.sync_info = mybir.SyncInfo(on_wait=[w], on_update=[])
        self.nc.sync.drain()

        self.nc.all_engine_barrier()
        assert self.sems is not None
        popped = self.nc._tile_sem_poison_stack.pop()
        assert popped is self._sem_poison
        self.nc.clear_and_free_semaphores(list(self.sems.allocated().values()))
        self.nc.all_engine_barrier()

    tilemod.TileContext._drain_and_barrier = _drain_and_barrier
    tilemod.TileContext._drain_patch_installed = True


def _split_sync_waits(nc, max_waits=1):
    """Hoist excess per-instruction sem waits onto same-engine NoOps."""
    import concourse.mybir as mybir

    n_added = 0
    for fn in nc.m.functions:
        for bb in fn.blocks:
            insts = bb.instructions
            new_list = []
            changed = False
            for inst in insts:
                si = inst.sync_info
                waits = list(si.on_wait) if si is not None and si.on_wait else []
                if (
                    len(waits) > max_waits
                    and inst.engine != mybir.EngineType.Unassigned
                ):
                    keep = waits[-max_waits:]
                    extra = waits[:-max_waits]
                    while extra:
                        chunk, extra = extra[:max_waits], extra[max_waits:]
                        nop = mybir.InstNoOp(
                            name=f"I-waitsplit-{n_added}", ins=[], outs=[]
                        )
                        nop.engine = inst.engine
                        nop.bass_nofuse = True
                        nop.sync_info = mybir.SyncInfo(on_wait=chunk, on_update=[])
                        new_list.append(nop)
                        n_added += 1
                    inst.sync_info = mybir.SyncInfo(
                        on_wait=keep, on_update=list(si.on_update or [])
                    )
                    changed = True
                new_list.append(inst)
            if changed:
                bb.instructions = new_list
    return n_added


# ---------------------------------------------------------------------------
# device program
# ---------------------------------------------------------------------------
def _build_program():
    import concourse.bass as bass
    import concourse.mybir as mybir
    import concourse.tile as tile

    f32 = mybir.dt.float32
    f32r = mybir.dt.float32r
    f16 = mybir.dt.float16
    Exp = mybir.ActivationFunctionType.Exp

    nc = bass.Bass("TRN2", target_bir_lowering=False, debug=False,
                   num_devices=N_CORES)

    xT = nc.dram_tensor("xT", [D_MODEL, SEQ], f32r, kind="ExternalInput").ap()
    wqT = nc.dram_tensor("wqT", [D_MODEL, 512], f32r, kind="ExternalInput").ap()
    wkT = nc.dram_tensor("wkT", [D_MODEL, 512], f32r, kind="ExternalInput").ap()
    wvT = nc.dram_tensor("wvT", [D_MODEL, 512], f32r, kind="ExternalInput").ap()
    woT = nc.dram_tensor("woT", [512, D_MODEL], f32r, kind="ExternalInput").ap()
    cos_d = nc.dram_tensor("cos_t", [128, SEQ], f32, kind="ExternalInput").ap()
    sin_d = nc.dram_tensor("sin_t", [128, SEQ], f32, kind="ExternalInput").ap()
    mask_d = nc.dram_tensor("bigmask", [128, 128], f32, kind="ExternalInput").ap()
    ones_d = nc.dram_tensor("ones64", [1, 64], f32r, kind="ExternalInput").ap()
    out_d = nc.dram_tensor("out", [SEQ, D_MODEL], f32, kind="ExternalOutput").ap()

    with tile.TileContext(nc) as tc:
        with (
            tc.tile_pool(name="consts", bufs=1) as cpool,
            tc.tile_pool(name="weights", bufs=3) as wpool,
            tc.tile_pool(name="csn", bufs=1) as cspool,
            tc.tile_pool(name="big", bufs=1) as big,
            tc.tile_pool(name="atp", bufs=2) as atpool,
            tc.tile_pool(name="xstream", bufs=2) as xpool,
            tc.tile_pool(name="rope", bufs=2) as rpool,
            tc.tile_pool(name="vtile", bufs=1) as vpool,
            tc.tile_pool(name="pt", bufs=2) as ppool,
            tc.tile_pool(name="small", bufs=2) as spool,
            tc.tile_pool(name="tiny", bufs=1) as tpool,
            tc.tile_pool(name="mm512", bufs=2, space="PSUM") as psA,
            tc.tile_pool(name="spsum", bufs=2, space="PSUM") as psS,
            tc.tile_pool(name="pvpsum", bufs=2, space="PSUM") as psV,
        ):
            # ---- constants into SBUF ----
            wq_sb = wpool.tile([128, KD * 512], f32r, tag="w", name="wq_sb")
            wk_sb = wpool.tile([128, KD * 512], f32r, tag="w", name="wk_sb")
            wv_sb = wpool.tile([128, KD * 512], f32r, tag="w", name="wv_sb")
            # W_Q first so the first projection chains can start while
            # W_K/W_V are still in flight (x tiles are queued in between,
            # from _pass_tiles below, before the wk/wv bulk)
            for k in range(KD):
                nc.sync.dma_start(wq_sb[:, k * 512:(k + 1) * 512],
                                  wqT[k * 128:(k + 1) * 128, :])
            mask_sb = cpool.tile([128, 128], f32, tag="mask")
            nc.sync.dma_start(mask_sb[:, :], mask_d[:, :])
            mask16 = cpool.tile([128, 128], f16, tag="mask16")
            nc.vector.tensor_copy(mask16[:, :], mask_sb[:, :])
            ones_sb = cpool.tile([1, 64], f32r, tag="ones")
            nc.sync.dma_start(ones_sb[:, :], ones_d[:, :])

            # ---- persistent activations ----
            qt_t = [big.tile([128, SEQ], f32r, tag=f"qt{c}", name=f"qt{c}")
                    for c in range(NCHUNK)]
            kt_t = [big.tile([128, SEQ], f32r, tag=f"kt{c}", name=f"kt{c}")
                    for c in range(NCHUNK)]
            # V in SBUF, fp16, [128, head, kblock, 64+ones]
            v_sb = vpool.tile([128, HPC * NSC * 65], f16, tag="vsb",
                              name="v_sb")
            v4 = v_sb.rearrange("p (h s n) -> p h s n", h=HPC, s=NSC)
            nc.vector.memset(v4[:, :, :, 64:65], 1.0)

            # ---- projection chain emitters (qt-major) ----
            pass_tiles = {}

            def _pass_tiles(qt):
                if qt not in pass_tiles:
                    xq = xpool.tile([128, KD * 512], f32r, tag="xq")
                    for k in range(KD):
                        nc.sync.dma_start(
                            xq[:, k * 512:(k + 1) * 512],
                            xT[k * 128:(k + 1) * 128, qt * 512:(qt + 1) * 512])
                    cs = cspool.tile([128, 512], f32, tag="cs")
                    sn = cspool.tile([128, 512], f32, tag="sn")
                    nc.sync.dma_start(cs[:, :],
                                      cos_d[:, qt * 512:(qt + 1) * 512])
                    nc.sync.dma_start(sn[:, :],
                                      sin_d[:, qt * 512:(qt + 1) * 512])
                    pass_tiles[qt] = (xq, cs, sn)
                return pass_tiles[qt]

            def emit_qk_chain(qt, wi, c):
                xq, cs, sn = _pass_tiles(qt)
                w_sb, dst = ((wq_sb, qt_t), (wk_sb, kt_t))[wi]
                ps = psA.tile([128, 512], f32, tag="mm512")
                for k in range(KD):
                    nc.tensor.matmul(
                        ps[:, :],
                        w_sb[:, k * 512 + c * 128:k * 512 + (c + 1) * 128],
                        xq[:, k * 512:(k + 1) * 512],
                        start=(k == 0), stop=(k == KD - 1))
                raw = rpool.tile([128, 512], f32, tag="raw", name="raw")
                nc.scalar.copy(raw[:, :], ps[:, :])
                rot = rpool.tile([128, 512], f32, tag="rot", name="rot")
                # swap 32-row blocks pairwise via SBUF->SBUF DMA on the
                # (otherwise idle) gpsimd trigger queue
                for q in range(4):
                    srow = (q // 2) * 64 + (1 - (q % 2)) * 32
                    nc.gpsimd.dma_start(rot[q * 32:(q + 1) * 32, :],
                                        raw[srow:srow + 32, :])
                dsl = dst[c][:, qt * 512:(qt + 1) * 512]
                nc.vector.tensor_mul(dsl, raw[:, :], cs[:, :])
                nc.vector.tensor_mul(rot[:, :], rot[:, :], sn[:, :])
                nc.vector.tensor_add(dsl, dsl, rot[:, :])

            def emit_v_chain(qt, scl):
                xq, _, _ = _pass_tiles(qt)
                sc = qt * 4 + scl
                ps = psA.tile([128, 512], f32, tag="mm512")
                for k in range(KD):
                    nc.tensor.matmul(
                        ps[:, :],
                        xq[:, k * 512 + scl * 128:k * 512 + (scl + 1) * 128],
                        wv_sb[:, k * 512:(k + 1) * 512],
                        start=(k == 0), stop=(k == KD - 1))
                ps3 = ps.rearrange("p (h n) -> p h n", h=HPC)
                nc.vector.tensor_copy(v4[:, :, sc, 0:64], ps3[:, :, :])

            def proj_pass_chains(qt):
                return ([(lambda qt=qt, wi=wi, c=c: emit_qk_chain(qt, wi, c))
                         for wi in (0, 1) for c in range(NCHUNK)] +
                        [(lambda qt=qt, scl=scl: emit_v_chain(qt, scl))
                         for scl in range(4)])

            # ---- out-projection chain emitters ----
            at_by_t = {}

            def emit_outproj(t, scl, nn):
                at_c = at_by_t[t]
                ps = psA.tile([128, 512], f32, tag="mm512")
                for kc in range(4):
                    nc.tensor.matmul(
                        ps[:, :],
                        at_c[kc][:, scl * 128:(scl + 1) * 128],
                        wo_sb[:, kc * D_MODEL + nn * 512:
                              kc * D_MODEL + (nn + 1) * 512],
                        start=(kc == 0), stop=(kc == 3))
                osb = spool.tile([128, 512], f32, tag="osb")
                nc.scalar.copy(osb[:, :], ps[:, :])
                nc.sync.dma_start(
                    out_d[(t * 4 + scl) * 128:(t * 4 + scl + 1) * 128,
                          nn * 512:(nn + 1) * 512],
                    osb[:, :])

            def outproj_chains(t):
                return [(lambda t=t, scl=scl, nn=nn: emit_outproj(t, scl, nn))
                        for scl in range(4) for nn in range(2)]

            # ---- attention steps (both heads of chunk c, key-block kb) ----
            def attention_step(t, c, kb, pv_pair, at_c):
                nkb = 4 * t + 4
                pvA, pvB = pv_pair
                jd = kb - 4 * t
                lo = 128 * jd if jd > 0 else 0
                sps = psS.tile([128, 1024], f32, tag="sps")
                # row-tiled pair: head 2c on PE rows 0-63, head 2c+1 on 64-127
                nc.tensor.matmul(
                    sps[:, lo:512],
                    kt_t[c][0:64, kb * 128:(kb + 1) * 128],
                    qt_t[c][0:64, t * 512 + lo:(t + 1) * 512],
                    start=True, stop=True)
                nc.tensor.matmul(
                    sps[:, 512 + lo:1024],
                    kt_t[c][64:128, kb * 128:(kb + 1) * 128],
                    qt_t[c][64:128, t * 512 + lo:(t + 1) * 512],
                    start=True, stop=True)
                pt = ppool.tile([128, 1024], f16, tag="pt")
                if lo == 0:
                    nc.scalar.activation(pt[:, :], sps[:, :], Exp)
                else:
                    sps2 = sps.rearrange("p (b n) -> p b n", b=2)
                    pt2 = pt.rearrange("p (b n) -> p b n", b=2)
                    nc.scalar.activation(pt2[:, :, lo:512],
                                         sps2[:, :, lo:512], Exp)
                if jd >= 0:
                    for half in range(2):
                        base = half * 512 + lo
                        nc.vector.tensor_mul(pt[:, base:base + 128],
                                             pt[:, base:base + 128],
                                             mask16[:, 0:128])
                nc.tensor.matmul(pvA[:, lo:512], v4[:, 2 * c, kb, :],
                                 pt[:, lo:512],
                                 start=(kb == 0), stop=(kb == nkb - 1))
                nc.tensor.matmul(pvB[:, lo:512], v4[:, 2 * c + 1, kb, :],
                                 pt[:, 512 + lo:1024],
                                 start=(kb == 0), stop=(kb == nkb - 1))

            den_by_t = {}

            def chunk_tail(t, c, pv_pair, at_c):
                # thin tail: move PV results out and bank the denominator
                # rows; normalization is deferred off the critical path
                pvA, pvB = pv_pair
                denX, denY = den_by_t[t]
                nc.scalar.copy(at_c[c][0:64, :], pvA[0:64, :])
                nc.vector.tensor_copy(at_c[c][64:128, :], pvB[0:64, :])
                nc.vector.tensor_copy(denX[32 * c:32 * c + 1, :],
                                      pvA[64:65, :])
                nc.vector.tensor_copy(denY[32 * c:32 * c + 1, :],
                                      pvB[64:65, :])

            def normalize_chains(t):
                # batched softmax normalization for tile t, run as fillers
                # during tile t+1: 2 batched reciprocals cover all 8 heads
                at_c = at_by_t[t]
                denX, denY = den_by_t[t]

                def recips():
                    nc.vector.reciprocal(denX[:, :], denX[:, :])
                    nc.vector.reciprocal(denY[:, :], denY[:, :])

                def head_chain(c, half):
                    den = (denX, denY)[half]
                    rec1 = tpool.tile([1, 512], f32r, tag=f"rec{half}",
                                      name=f"rec{half}_{t}_{c}")
                    nc.vector.tensor_copy(rec1[:, :],
                                          den[32 * c:32 * c + 1, :])
                    bps = psA.tile([64, 512], f32, tag="mm512",
                                   name=f"bps{half}_{t}_{c}")
                    nc.tensor.matmul(bps[:, :], ones_sb[:, :], rec1[:, :],
                                     start=True, stop=True)
                    r0 = half * 64
                    nc.vector.tensor_mul(at_c[c][r0:r0 + 64, :],
                                         at_c[c][r0:r0 + 64, :], bps[:, :])

                return [recips] + [
                    (lambda c=c, half=half: head_chain(c, half))
                    for c in range(NCHUNK) for half in (0, 1)]

            # ---- schedule: proj passes 0,1 upfront; attention tile t
            # interleaved with proj pass t+2 / out-proj fillers ----
            with nc.named_scope("qkv_proj"):
                # x/cos/sin for the first two passes queue ahead of the
                # wk/wv bulk so the first Q chains start ~11us earlier
                _pass_tiles(0)
                _pass_tiles(1)
                for w_sb, w_d in ((wk_sb, wkT), (wv_sb, wvT)):
                    for k in range(KD):
                        nc.sync.dma_start(w_sb[:, k * 512:(k + 1) * 512],
                                          w_d[k * 128:(k + 1) * 128, :])
                for chain in proj_pass_chains(0):
                    chain()
                for chain in proj_pass_chains(1):
                    chain()

            wo_sb = wpool.tile([128, 4 * D_MODEL], f32r, tag="w",
                               name="wo_sb")
            for k in range(4):
                nc.sync.dma_start(
                    wo_sb[:, k * D_MODEL:(k + 1) * D_MODEL],
                    woT[k * 128:(k + 1) * 128, :])

            with nc.named_scope("attention"):
                for t in range(NQT):
                    at_c = [atpool.tile([128, 512], f32r, tag=f"at{c}",
                                        name=f"at{c}_{t}")
                            for c in range(NCHUNK)]
                    at_by_t[t] = at_c
                    den_by_t[t] = (
                        tpool.tile([128, 512], f32, tag="denX",
                                   name=f"denX_{t}"),
                        tpool.tile([128, 512], f32, tag="denY",
                                   name=f"denY_{t}"))
                    nkb = 4 * t + 4
                    steps = [(c, kb) for c in range(NCHUNK)
                             for kb in range(nkb)]
                    # fillers: evenly-spread proj/normalize chains; out-proj
                    # chains front-loaded (wo/at deps are ready then)
                    spread, front = [], []
                    if t + 2 < NQT:
                        spread = proj_pass_chains(t + 2)
                    if t >= 1:
                        # single-buffered den tiles: all of tile t-1's
                        # normalize reads must precede tile t's first tail
                        front = front + normalize_chains(t - 1)
                    if t == 2:
                        front = front + outproj_chains(0)
                    elif t == 3:
                        # outproj(1) must precede at(3) writes (slot reuse);
                        # outproj(2)'s slot is never reused, so it can
                        # spread out and fill the filler-free late steps
                        front = front + outproj_chains(1)
                        spread = spread + outproj_chains(2)
                    fill_at = {}
                    for fi, chain in enumerate(spread):
                        si = min(len(steps) - 1,
                                 (fi + 1) * len(steps) // (len(spread) + 1))
                        fill_at.setdefault(si, []).append(chain)
                    # out-proj chains run before any of this tile's attention
                    # steps: their at(t-2) reads must precede at(t) writes
                    for chain in front:
                        chain()
                    pv_pair = None
                    for si, (c, kb) in enumerate(steps):
                        for chain in fill_at.get(si, ()):
                            chain()
                        if kb == 0:
                            pv_pair = (psV.tile([65, 512], f32, tag="pv",
                                                name=f"pvA_{t}_{c}"),
                                       psV.tile([65, 512], f32, tag="pv",
                                                name=f"pvB_{t}_{c}"))
                        attention_step(t, c, kb, pv_pair, at_c)
                        if kb == nkb - 1:
                            chunk_tail(t, c, pv_pair, at_c)
                for chain in normalize_chains(3):
                    chain()
                for chain in outproj_chains(3):
                    chain()

    return nc


# ---------------------------------------------------------------------------
# host side
# ---------------------------------------------------------------------------
_PROG_CACHE = {}


def _get_program():
    if "nc" not in _PROG_CACHE:
        _install_axon_hooks()
        _install_drain_patch()
        _PROG_CACHE["nc"] = _build_program()
    return _PROG_CACHE["nc"]


def _prep_in_maps(inputs):
    x = np.asarray(inputs["x"], np.float32)
    pos = np.asarray(inputs["token_positions"]).astype(np.float32)
    WQ = np.asarray(inputs["W_Q"], np.float32)
    WK = np.asarray(inputs["W_K"], np.float32)
    WV = np.asarray(inputs["W_V"], np.float32)
    WO = np.asarray(inputs["W_O"], np.float32)

    # NeoX reorder of interleaved rope pairs, per head (rows of W_Q/W_K)
    perm = np.empty(D_MODEL, np.int64)
    for h in range(NUM_HEADS):
        b = h * HEAD_DIM
        perm[b:b + 32] = b + 2 * np.arange(32)
        perm[b + 32:b + 64] = b + 2 * np.arange(32) + 1
    WQp = WQ[perm] * np.float32(HEAD_DIM ** -0.5)
    WKp = WK[perm]

    # rope tables, mirroring the reference's float32 math
    j = np.arange(HEAD_DIM // 2, dtype=np.float32)
    inv_freq = np.power(np.float32(THETA),
                        (np.float32(-2.0) * j / np.float32(HEAD_DIM))
                        ).astype(np.float32)
    ang = pos[:, None] * inv_freq[None, :]          # (SEQ, 32) f32
    cos = np.cos(ang).astype(np.float32).T          # (32, SEQ)
    sin = np.sin(ang).astype(np.float32).T
    cos_t = np.ascontiguousarray(np.tile(cos, (4, 1)))           # (128, SEQ)
    sin_t = np.ascontiguousarray(
        np.concatenate([-sin, sin, -sin, sin], axis=0))          # (128, SEQ)

    tri = (np.arange(128)[:, None] <= np.arange(128)[None, :])
    bigmask = tri.astype(np.float32)
    ones64 = np.ones((1, 64), np.float32)

    in_maps = []
    for core in range(N_CORES):
        b, g = core // 2, core % 2
        sl = slice(g * 512, (g + 1) * 512)
        in_maps.append({
            "xT": np.ascontiguousarray(x[b].T),
            "wqT": np.ascontiguousarray(WQp[sl].T),
            "wkT": np.ascontiguousarray(WKp[sl].T),
            "wvT": np.ascontiguousarray(WV[sl].T),
            "woT": np.ascontiguousarray(WO[:, sl].T),
            "cos_t": cos_t,
            "sin_t": sin_t,
            "bigmask": bigmask,
            "ones64": ones64,
        })
    return in_maps


def kernel(**inputs):
    from concourse.bass_utils import run_bass_kernel_spmd

    nc = _get_program()
    if not _PROG_CACHE.get("waits_split"):
        _split_sync_waits(nc)
        _PROG_CACHE["waits_split"] = True
    in_maps = _prep_in_maps(inputs)
    trace = os.environ.get("BASS_KERNEL_TRACE") == "1"
    kw = {}
    if trace:
        kw = dict(trace=True, tmpdir=os.environ.get("BASS_KERNEL_TRACE_DIR"))
    res = run_bass_kernel_spmd(nc, in_maps, core_ids=list(range(N_CORES)), **kw)
    if trace:
        print(f"HW exec time: {res.exec_time_ns} ns "
              f"(mean {res.mean_exec_time_ns}, "
              f"max core {res.max_exec_time_core_id})")
        _PROG_CACHE["last_results"] = res

    out = np.empty((BATCH, SEQ, D_MODEL), np.float32)
    for b in range(BATCH):
        out[b] = res.results[2 * b]["out"] + res.results[2 * b + 1]["out"]
    return out


# revision 33
# speedup vs baseline: 1.1214x; 1.1214x over previous
"""Multi-head attention (RoPE + causal softmax) Trainium2 Bass kernel.

Sharding: 8 cores = 4 batches x 2 head-groups (tensor-parallel over heads).
Each core computes, for its (batch b, head-group g):
    Q/K/V projections for its 8 heads, RoPE, causal attention, and a
    partial output projection with its 512-row slice of W_O^T.
Host sums the two partial outputs per batch.

v2 restructure (vs the 811us baseline):
  - RoPE rotate-halves now: ACT copy PSUM->SBUF, 4 SBUF->SBUF DMA
    partition-block swaps, then 2 DVE muls + 1 add. (The old gpsimd
    copies were 1.86us each and serialized the projection phase.)
  - V stays in SBUF as fp16 with the ones-column baked in (layout
    [128, head, kblock, 65]); no DRAM round trip.
  - P (exp scores) in fp16: exp max here is ~13 so fp16 is safe and
    keeps ~2e-4 accuracy; halves mask-mul cost and pt SBUF footprint.
  - Attention is t-outer / head-inner, so attention on query-tile t only
    depends on projection pass t: the Tile scheduler overlaps proj pass
    t+1 with attention tile t, keeping the PE dense (HAM stays warm).
  - Normalization inline per (head, t): reciprocal straight from the
    PV PSUM ones-row, broadcast via a tiny ones-matmul.
  - Output projection runs per query-tile right after its 8 heads.
"""

import os
import sys
import types

import numpy as np

D_MODEL = 1024
NUM_HEADS = 16
HEAD_DIM = 64
THETA = 10000.0
BATCH = 4
SEQ = 2048
N_CORES = 8
HPC = 8          # heads per core
NCHUNK = HPC // 2  # 128-row chunks of the per-core 512 head dims
NQT = SEQ // 512   # 512-wide seq tiles
NSC = SEQ // 128   # 128-row seq chunks
KD = D_MODEL // 128  # contraction chunks for projections


# ---------------------------------------------------------------------------
# environment shims (axon container: missing antenv.axon_hooks; walrus here
# supports only 1 sync-wait per instruction)
# ---------------------------------------------------------------------------
def _install_axon_hooks():
    import antenv

    if hasattr(antenv, "axon_hooks"):
        return
    mod = types.ModuleType("antenv.axon_hooks")
    _hook = [None]
    mod.set_axon_ntff_profile_hook = lambda h: _hook.__setitem__(0, h)
    mod.get_axon_ntff_profile_hook = lambda: _hook[0]
    sys.modules["antenv.axon_hooks"] = mod
    antenv.axon_hooks = mod
    try:
        from trn_agent_boot.trn_boot import _ntff_profile_via_ctypes

        mod.set_axon_ntff_profile_hook(
            _ntff_profile_via_ctypes("/opt/axon/libaxon_pjrt.so")
        )
    except Exception:
        pass


def _install_drain_patch():
    import concourse.mybir as mybir
    import concourse.tile as tilemod

    if getattr(tilemod.TileContext, "_drain_patch_installed", False):
        return

    def _drain_and_barrier(self, tick_clock, wait_clock):
        carrier = self.nc.sync.nop(nofuse=True)
        wait_clock.add_sem_waits(
            carrier.ins, tilemod.ScopedClock({None: tick_clock.global_clock})
        )
        si = carrier.ins.sync_info
        if si is not None and si.on_wait and len(si.on_wait) > 1:
            waits = list(si.on_wait)
            carrier.ins.sync_info = mybir.SyncInfo(
                on_wait=[waits[0]], on_update=list(si.on_update or [])
            )
            for w in waits[1:]:
                nop = self.nc.sync.nop(nofuse=True)
                nop.ins.sync_info = mybir.SyncInfo(on_wait=[w], on_update=[])
        self.nc.sync.drain()

        self.nc.all_engine_barrier()
        assert self.sems is not None
        popped = self.nc._tile_sem_poison_stack.pop()
        assert popped is self._sem_poison
        self.nc.clear_and_free_semaphores(list(self.sems.allocated().values()))
        self.nc.all_engine_barrier()

    tilemod.TileContext._drain_and_barrier = _drain_and_barrier
    tilemod.TileContext._drain_patch_installed = True


def _split_sync_waits(nc, max_waits=1):
    """Hoist excess per-instruction sem waits onto same-engine NoOps."""
    import concourse.mybir as mybir

    n_added = 0
    for fn in nc.m.functions:
        for bb in fn.blocks:
            insts = bb.instructions
            new_list = []
            changed = False
            for inst in insts:
                si = inst.sync_info
                waits = list(si.on_wait) if si is not None and si.on_wait else []
                if (
                    len(waits) > max_waits
                    and inst.engine != mybir.EngineType.Unassigned
                ):
                    keep = waits[-max_waits:]
                    extra = waits[:-max_waits]
                    while extra:
                        chunk, extra = extra[:max_waits], extra[max_waits:]
                        nop = mybir.InstNoOp(
                            name=f"I-waitsplit-{n_added}", ins=[], outs=[]
                        )
                        nop.engine = inst.engine
                        nop.bass_nofuse = True
                        nop.sync_info = mybir.SyncInfo(on_wait=chunk, on_update=[])
                        new_list.append(nop)
                        n_added += 1
                    inst.sync_info = mybir.SyncInfo(
                        on_wait=keep, on_update=list(si.on_update or [])
                    )
                    changed = True
                new_list.append(inst)
            if changed:
                bb.instructions = new_list
    return n_added


# ---------------------------------------------------------------------------
# device program
# ---------------------------------------------------------------------------
def _build_program():
    import concourse.bass as bass
    import concourse.mybir as mybir
    import concourse.tile as tile

    f32 = mybir.dt.float32
    f32r = mybir.dt.float32r
    f16 = mybir.dt.float16
    Exp = mybir.ActivationFunctionType.Exp

    nc = bass.Bass("TRN2", target_bir_lowering=False, debug=False,
                   num_devices=N_CORES)

    xT = nc.dram_tensor("xT", [D_MODEL, SEQ], f32r, kind="ExternalInput").ap()
    wqT = nc.dram_tensor("wqT", [D_MODEL, 512], f32r, kind="ExternalInput").ap()
    wkT = nc.dram_tensor("wkT", [D_MODEL, 512], f32r, kind="ExternalInput").ap()
    wvT = nc.dram_tensor("wvT", [D_MODEL, 512], f32r, kind="ExternalInput").ap()
    woT = nc.dram_tensor("woT", [512, D_MODEL], f32r, kind="ExternalInput").ap()
    cos_d = nc.dram_tensor("cos_t", [128, SEQ], f32, kind="ExternalInput").ap()
    sin_d = nc.dram_tensor("sin_t", [128, SEQ], f32, kind="ExternalInput").ap()
    mask_d = nc.dram_tensor("bigmask", [128, 128], f32, kind="ExternalInput").ap()
    ones_d = nc.dram_tensor("ones64", [1, 64], f32r, kind="ExternalInput").ap()
    out_d = nc.dram_tensor("out", [SEQ, D_MODEL], f32, kind="ExternalOutput").ap()

    with tile.TileContext(nc) as tc:
        with (
            tc.tile_pool(name="consts", bufs=1) as cpool,
            tc.tile_pool(name="weights", bufs=3) as wpool,
            tc.tile_pool(name="csn", bufs=1) as cspool,
            tc.tile_pool(name="big", bufs=1) as big,
            tc.tile_pool(name="atp", bufs=2) as atpool,
            tc.tile_pool(name="xstream", bufs=2) as xpool,
            tc.tile_pool(name="rope", bufs=2) as rpool,
            tc.tile_pool(name="vtile", bufs=1) as vpool,
            tc.tile_pool(name="pt", bufs=2) as ppool,
            tc.tile_pool(name="small", bufs=2) as spool,
            tc.tile_pool(name="tiny", bufs=1) as tpool,
            tc.tile_pool(name="mm512", bufs=2, space="PSUM") as psA,
            tc.tile_pool(name="spsum", bufs=2, space="PSUM") as psS,
            tc.tile_pool(name="pvpsum", bufs=2, space="PSUM") as psV,
        ):
            # ---- constants into SBUF ----
            wq_sb = wpool.tile([128, KD * 512], f32r, tag="w", name="wq_sb")
            wk_sb = wpool.tile([128, KD * 512], f32r, tag="w", name="wk_sb")
            wv_sb = wpool.tile([128, KD * 512], f32r, tag="w", name="wv_sb")
            # W_Q on the sync queue; W_K/W_V ride the (startup-idle) gpsimd
            # trigger queue so pass-0's x tiles aren't queued behind them
            for k in range(KD):
                nc.sync.dma_start(wq_sb[:, k * 512:(k + 1) * 512],
                                  wqT[k * 128:(k + 1) * 128, :])
            for w_sb, w_d in ((wk_sb, wkT), (wv_sb, wvT)):
                for k in range(KD):
                    nc.gpsimd.dma_start(w_sb[:, k * 512:(k + 1) * 512],
                                        w_d[k * 128:(k + 1) * 128, :])
            mask_sb = cpool.tile([128, 128], f32, tag="mask")
            nc.sync.dma_start(mask_sb[:, :], mask_d[:, :])
            mask16 = cpool.tile([128, 128], f16, tag="mask16")
            nc.vector.tensor_copy(mask16[:, :], mask_sb[:, :])
            ones_sb = cpool.tile([1, 64], f32r, tag="ones")
            nc.sync.dma_start(ones_sb[:, :], ones_d[:, :])

            # ---- persistent activations ----
            qt_t = [big.tile([128, SEQ], f32r, tag=f"qt{c}", name=f"qt{c}")
                    for c in range(NCHUNK)]
            kt_t = [big.tile([128, SEQ], f32r, tag=f"kt{c}", name=f"kt{c}")
                    for c in range(NCHUNK)]
            # V in SBUF, fp16, [128, head, kblock, 64+ones]
            v_sb = vpool.tile([128, HPC * NSC * 65], f16, tag="vsb",
                              name="v_sb")
            v4 = v_sb.rearrange("p (h s n) -> p h s n", h=HPC, s=NSC)
            nc.vector.memset(v4[:, :, :, 64:65], 1.0)

            # ---- projection chain emitters (qt-major) ----
            pass_tiles = {}

            def _pass_tiles(qt):
                if qt not in pass_tiles:
                    xq = xpool.tile([128, KD * 512], f32r, tag="xq")
                    for k in range(KD):
                        nc.sync.dma_start(
                            xq[:, k * 512:(k + 1) * 512],
                            xT[k * 128:(k + 1) * 128, qt * 512:(qt + 1) * 512])
                    cs = cspool.tile([128, 512], f32, tag="cs")
                    sn = cspool.tile([128, 512], f32, tag="sn")
                    nc.sync.dma_start(cs[:, :],
                                      cos_d[:, qt * 512:(qt + 1) * 512])
                    nc.sync.dma_start(sn[:, :],
                                      sin_d[:, qt * 512:(qt + 1) * 512])
                    pass_tiles[qt] = (xq, cs, sn)
                return pass_tiles[qt]

            def emit_qk_chain(qt, wi, c):
                xq, cs, sn = _pass_tiles(qt)
                w_sb, dst = ((wq_sb, qt_t), (wk_sb, kt_t))[wi]
                ps = psA.tile([128, 512], f32, tag="mm512")
                for k in range(KD):
                    nc.tensor.matmul(
                        ps[:, :],
                        w_sb[:, k * 512 + c * 128:k * 512 + (c + 1) * 128],
                        xq[:, k * 512:(k + 1) * 512],
                        start=(k == 0), stop=(k == KD - 1))
                raw = rpool.tile([128, 512], f32, tag="raw", name="raw")
                nc.scalar.copy(raw[:, :], ps[:, :])
                rot = rpool.tile([128, 512], f32, tag="rot", name="rot")
                # swap 32-row blocks pairwise via SBUF->SBUF DMA on the
                # (otherwise idle) gpsimd trigger queue
                for q in range(4):
                    srow = (q // 2) * 64 + (1 - (q % 2)) * 32
                    nc.gpsimd.dma_start(rot[q * 32:(q + 1) * 32, :],
                                        raw[srow:srow + 32, :])
                dsl = dst[c][:, qt * 512:(qt + 1) * 512]
                nc.vector.tensor_mul(dsl, raw[:, :], cs[:, :])
                nc.vector.tensor_mul(rot[:, :], rot[:, :], sn[:, :])
                nc.vector.tensor_add(dsl, dsl, rot[:, :])

            def emit_v_chain(qt, scl):
                xq, _, _ = _pass_tiles(qt)
                sc = qt * 4 + scl
                ps = psA.tile([128, 512], f32, tag="mm512")
                for k in range(KD):
                    nc.tensor.matmul(
                        ps[:, :],
                        xq[:, k * 512 + scl * 128:k * 512 + (scl + 1) * 128],
                        wv_sb[:, k * 512:(k + 1) * 512],
                        start=(k == 0), stop=(k == KD - 1))
                ps3 = ps.rearrange("p (h n) -> p h n", h=HPC)
                nc.vector.tensor_copy(v4[:, :, sc, 0:64], ps3[:, :, :])

            def proj_pass_chains(qt):
                return ([(lambda qt=qt, wi=wi, c=c: emit_qk_chain(qt, wi, c))
                         for wi in (0, 1) for c in range(NCHUNK)] +
                        [(lambda qt=qt, scl=scl: emit_v_chain(qt, scl))
                         for scl in range(4)])

            # ---- out-projection chain emitters ----
            at_by_t = {}

            def emit_outproj(t, scl, nn):
                at_c = at_by_t[t]
                ps = psA.tile([128, 512], f32, tag="mm512")
                for kc in range(4):
                    nc.tensor.matmul(
                        ps[:, :],
                        at_c[kc][:, scl * 128:(scl + 1) * 128],
                        wo_sb[:, kc * D_MODEL + nn * 512:
                              kc * D_MODEL + (nn + 1) * 512],
                        start=(kc == 0), stop=(kc == 3))
                osb = spool.tile([128, 512], f32, tag="osb")
                nc.scalar.copy(osb[:, :], ps[:, :])
                nc.sync.dma_start(
                    out_d[(t * 4 + scl) * 128:(t * 4 + scl + 1) * 128,
                          nn * 512:(nn + 1) * 512],
                    osb[:, :])

            def outproj_chains(t):
                return [(lambda t=t, scl=scl, nn=nn: emit_outproj(t, scl, nn))
                        for scl in range(4) for nn in range(2)]

            # ---- attention steps (both heads of chunk c, key-block kb),
            # software-pipelined: PV for block kb is emitted one iteration
            # after its scores/exp, so the PE streams scores(kb+1) while
            # ACT computes exp(kb) instead of stalling ----
            def attention_scores(t, c, kb):
                jd = kb - 4 * t
                lo = 128 * jd if jd > 0 else 0
                sps = psS.tile([128, 1024], f32, tag="sps")
                # row-tiled pair: head 2c on PE rows 0-63, head 2c+1 on 64-127
                nc.tensor.matmul(
                    sps[:, lo:512],
                    kt_t[c][0:64, kb * 128:(kb + 1) * 128],
                    qt_t[c][0:64, t * 512 + lo:(t + 1) * 512],
                    start=True, stop=True)
                nc.tensor.matmul(
                    sps[:, 512 + lo:1024],
                    kt_t[c][64:128, kb * 128:(kb + 1) * 128],
                    qt_t[c][64:128, t * 512 + lo:(t + 1) * 512],
                    start=True, stop=True)
                pt = ppool.tile([128, 1024], f16, tag="pt")
                if lo == 0:
                    nc.scalar.activation(pt[:, :], sps[:, :], Exp)
                else:
                    sps2 = sps.rearrange("p (b n) -> p b n", b=2)
                    pt2 = pt.rearrange("p (b n) -> p b n", b=2)
                    nc.scalar.activation(pt2[:, :, lo:512],
                                         sps2[:, :, lo:512], Exp)
                if jd >= 0:
                    for half in range(2):
                        base = half * 512 + lo
                        nc.vector.tensor_mul(pt[:, base:base + 128],
                                             pt[:, base:base + 128],
                                             mask16[:, 0:128])
                return pt, lo

            def attention_pv(t, c, kb, pt, lo, pv_pair):
                nkb = 4 * t + 4
                pvA, pvB = pv_pair
                nc.tensor.matmul(pvA[:, lo:512], v4[:, 2 * c, kb, :],
                                 pt[:, lo:512],
                                 start=(kb == 0), stop=(kb == nkb - 1))
                nc.tensor.matmul(pvB[:, lo:512], v4[:, 2 * c + 1, kb, :],
                                 pt[:, 512 + lo:1024],
                                 start=(kb == 0), stop=(kb == nkb - 1))

            den_by_t = {}

            def chunk_tail(t, c, pv_pair, at_c):
                # thin tail: move PV results out and bank the denominator
                # rows; normalization is deferred off the critical path
                pvA, pvB = pv_pair
                denX, denY = den_by_t[t]
                nc.scalar.copy(at_c[c][0:64, :], pvA[0:64, :])
                nc.vector.tensor_copy(at_c[c][64:128, :], pvB[0:64, :])
                nc.vector.tensor_copy(denX[32 * c:32 * c + 1, :],
                                      pvA[64:65, :])
                nc.vector.tensor_copy(denY[32 * c:32 * c + 1, :],
                                      pvB[64:65, :])

            def normalize_chains(t):
                # batched softmax normalization for tile t, run as fillers
                # during tile t+1: 2 batched reciprocals cover all 8 heads
                at_c = at_by_t[t]
                denX, denY = den_by_t[t]

                def recips():
                    nc.vector.reciprocal(denX[:, :], denX[:, :])
                    nc.vector.reciprocal(denY[:, :], denY[:, :])

                def head_chain(c, half):
                    den = (denX, denY)[half]
                    rec1 = tpool.tile([1, 512], f32r, tag=f"rec{half}",
                                      name=f"rec{half}_{t}_{c}")
                    nc.vector.tensor_copy(rec1[:, :],
                                          den[32 * c:32 * c + 1, :])
                    bps = psA.tile([64, 512], f32, tag="mm512",
                                   name=f"bps{half}_{t}_{c}")
                    nc.tensor.matmul(bps[:, :], ones_sb[:, :], rec1[:, :],
                                     start=True, stop=True)
                    r0 = half * 64
                    nc.vector.tensor_mul(at_c[c][r0:r0 + 64, :],
                                         at_c[c][r0:r0 + 64, :], bps[:, :])

                return [recips] + [
                    (lambda c=c, half=half: head_chain(c, half))
                    for c in range(NCHUNK) for half in (0, 1)]

            # ---- schedule: proj passes 0,1 upfront; attention tile t
            # interleaved with proj pass t+2 / out-proj fillers ----
            with nc.named_scope("qkv_proj"):
                for chain in proj_pass_chains(0):
                    chain()
                for chain in proj_pass_chains(1):
                    chain()

            wo_sb = wpool.tile([128, 4 * D_MODEL], f32r, tag="w",
                               name="wo_sb")
            for k in range(4):
                nc.sync.dma_start(
                    wo_sb[:, k * D_MODEL:(k + 1) * D_MODEL],
                    woT[k * 128:(k + 1) * 128, :])

            with nc.named_scope("attention"):
                for t in range(NQT):
                    at_c = [atpool.tile([128, 512], f32r, tag=f"at{c}",
                                        name=f"at{c}_{t}")
                            for c in range(NCHUNK)]
                    at_by_t[t] = at_c
                    den_by_t[t] = (
                        tpool.tile([128, 512], f32, tag="denX",
                                   name=f"denX_{t}"),
                        tpool.tile([128, 512], f32, tag="denY",
                                   name=f"denY_{t}"))
                    nkb = 4 * t + 4
                    steps = [(c, kb) for c in range(NCHUNK)
                             for kb in range(nkb)]
                    # fillers: evenly-spread proj/normalize chains; out-proj
                    # chains front-loaded (wo/at deps are ready then)
                    spread, front = [], []
                    if t + 2 < NQT:
                        spread = proj_pass_chains(t + 2)
                    if t >= 1:
                        # single-buffered den tiles: all of tile t-1's
                        # normalize reads must precede tile t's first tail
                        front = front + normalize_chains(t - 1)
                    if t == 2:
                        front = front + outproj_chains(0)
                    elif t == 3:
                        # outproj(1) must precede at(3) writes (slot reuse);
                        # outproj(2)'s slot is never reused, so it can
                        # spread out and fill the filler-free late steps
                        front = front + outproj_chains(1)
                        spread = spread + outproj_chains(2)
                    fill_at = {}
                    for fi, chain in enumerate(spread):
                        si = min(len(steps) - 1,
                                 (fi + 1) * len(steps) // (len(spread) + 1))
                        fill_at.setdefault(si, []).append(chain)
                    # out-proj chains run before any of this tile's attention
                    # steps: their at(t-2) reads must precede at(t) writes
                    for chain in front:
                        chain()
                    pv_pair = None
                    pending = None
                    for si, (c, kb) in enumerate(steps):
                        for chain in fill_at.get(si, ()):
                            chain()
                        if kb == 0:
                            pv_pair = (psV.tile([65, 512], f32, tag="pv",
                                                name=f"pvA_{t}_{c}"),
                                       psV.tile([65, 512], f32, tag="pv",
                                                name=f"pvB_{t}_{c}"))
                        pt, lo = attention_scores(t, c, kb)
                        if pending is not None:
                            attention_pv(t, c, pending[0], pending[1],
                                         pending[2], pv_pair)
                        pending = (kb, pt, lo)
                        if kb == nkb - 1:
                            attention_pv(t, c, kb, pt, lo, pv_pair)
                            pending = None
                            chunk_tail(t, c, pv_pair, at_c)
                for chain in normalize_chains(3):
                    chain()
                for chain in outproj_chains(3):
                    chain()

    return nc


# ---------------------------------------------------------------------------
# host side
# ---------------------------------------------------------------------------
_PROG_CACHE = {}


def _get_program():
    if "nc" not in _PROG_CACHE:
        _install_axon_hooks()
        _install_drain_patch()
        _PROG_CACHE["nc"] = _build_program()
    return _PROG_CACHE["nc"]


def _prep_in_maps(inputs):
    x = np.asarray(inputs["x"], np.float32)
    pos = np.asarray(inputs["token_positions"]).astype(np.float32)
    WQ = np.asarray(inputs["W_Q"], np.float32)
    WK = np.asarray(inputs["W_K"], np.float32)
    WV = np.asarray(inputs["W_V"], np.float32)
    WO = np.asarray(inputs["W_O"], np.float32)

    # NeoX reorder of interleaved rope pairs, per head (rows of W_Q/W_K)
    perm = np.empty(D_MODEL, np.int64)
    for h in range(NUM_HEADS):
        b = h * HEAD_DIM
        perm[b:b + 32] = b + 2 * np.arange(32)
        perm[b + 32:b + 64] = b + 2 * np.arange(32) + 1
    WQp = WQ[perm] * np.float32(HEAD_DIM ** -0.5)
    WKp = WK[perm]

    # rope tables, mirroring the reference's float32 math
    j = np.arange(HEAD_DIM // 2, dtype=np.float32)
    inv_freq = np.power(np.float32(THETA),
                        (np.float32(-2.0) * j / np.float32(HEAD_DIM))
                        ).astype(np.float32)
    ang = pos[:, None] * inv_freq[None, :]          # (SEQ, 32) f32
    cos = np.cos(ang).astype(np.float32).T          # (32, SEQ)
    sin = np.sin(ang).astype(np.float32).T
    cos_t = np.ascontiguousarray(np.tile(cos, (4, 1)))           # (128, SEQ)
    sin_t = np.ascontiguousarray(
        np.concatenate([-sin, sin, -sin, sin], axis=0))          # (128, SEQ)

    tri = (np.arange(128)[:, None] <= np.arange(128)[None, :])
    bigmask = tri.astype(np.float32)
    ones64 = np.ones((1, 64), np.float32)

    in_maps = []
    for core in range(N_CORES):
        b, g = core // 2, core % 2
        sl = slice(g * 512, (g + 1) * 512)
        in_maps.append({
            "xT": np.ascontiguousarray(x[b].T),
            "wqT": np.ascontiguousarray(WQp[sl].T),
            "wkT": np.ascontiguousarray(WKp[sl].T),
            "wvT": np.ascontiguousarray(WV[sl].T),
            "woT": np.ascontiguousarray(WO[:, sl].T),
            "cos_t": cos_t,
            "sin_t": sin_t,
            "bigmask": bigmask,
            "ones64": ones64,
        })
    return in_maps


def kernel(**inputs):
    from concourse.bass_utils import run_bass_kernel_spmd

    nc = _get_program()
    if not _PROG_CACHE.get("waits_split"):
        _split_sync_waits(nc)
        _PROG_CACHE["waits_split"] = True
    in_maps = _prep_in_maps(inputs)
    trace = os.environ.get("BASS_KERNEL_TRACE") == "1"
    kw = {}
    if trace:
        kw = dict(trace=True, tmpdir=os.environ.get("BASS_KERNEL_TRACE_DIR"))
    res = run_bass_kernel_spmd(nc, in_maps, core_ids=list(range(N_CORES)), **kw)
    if trace:
        print(f"HW exec time: {res.exec_time_ns} ns "
              f"(mean {res.mean_exec_time_ns}, "
              f"max core {res.max_exec_time_core_id})")
        _PROG_CACHE["last_results"] = res

    out = np.empty((BATCH, SEQ, D_MODEL), np.float32)
    for b in range(BATCH):
        out[b] = res.results[2 * b]["out"] + res.results[2 * b + 1]["out"]
    return out
